# revision 13
# baseline (speedup 1.0000x reference)
"""AttentionBlock (diffusion-UNet style) on 8 TRN2 NeuronCores.

Sharding: data-parallel over batch N=8 (one batch element per core), all
params replicated. BatchNorm statistics couple the batch, so each core
computes per-channel (sum, sumsq) over its own element and a tiny (2 KB)
AllReduce produces the global stats.

Per-core layout trick: everything stays channel-major [C, pos] with
pos = h*32 + w (the natural memory order of x). Attention is invariant to a
consistent relabeling of query+key positions, so the reference's
transpose(1,3) never has to be materialized:
  Q_T  [512,1024] = Wq.T @ xn      (lhsT=Wq, rhs=xn — both natural layouts)
  K_T  [512,1024] = Wk.T @ xn
  S_T_h[1024,1024] = K_T_h.T @ Q_T_h   (scores, transposed: keys on partitions)
  P_T  = exp(S_T / sqrt(512))          (no max-subtraction: |scores| < ~3)
  AV_h [65,1024]  = [V | 1].T @ P_T    (ones column makes row 64 the softmax
                                        denominator — free: matmul time only
                                        depends on the moving dim)
  A    [512,1024] (head-major rows) = AV rows / denominator
  OUT_T[256,1024] = Wp_perm.T @ A + bp + x   (Wp rows permuted on host from
                                              dv-major to head-major)

All matmuls run in bf16 (1 cycle/row on the PE vs 4 for fp32); accumulation
is fp32 in PSUM, and BN/softmax-scale arithmetic stays fp32.
"""

import os
import numpy as np

N_CORES = 8
C = 256          # d_in == d_out
POS = 1024       # 32*32 spatial positions
DKQ = 512
DV = 64
H = 8
DH = DKQ // H    # 64
TD = 512         # t embedding dim
EPS = 1e-5
SCALE = 1.0 / float(np.sqrt(np.float32(DKQ)))
CNT = float(N_CORES * POS)  # 8192 elements per channel for BN stats

_CACHE = {}


def _build_program():
    import concourse.bass as bass
    import concourse.tile as tile
    from concourse import bacc, mybir

    f32 = mybir.dt.float32
    bf16 = mybir.dt.bfloat16
    AF = mybir.ActivationFunctionType
    ALU = mybir.AluOpType

    nc = bacc.Bacc(
        "TRN2",
        target_bir_lowering=False,
        debug=False,
        num_devices=N_CORES,
    )

    # Per-core DRAM I/O (host pre-reshapes everything to [chunks, 128, free];
    # weight-like matmul inputs arrive as bf16).
    x_d = nc.dram_tensor("x", [2, 128, POS], f32, kind="ExternalInput").ap()
    t_d = nc.dram_tensor("t", [4, 128, 1], bf16, kind="ExternalInput").ap()
    wt_d = nc.dram_tensor("wt", [4, 128, C], bf16, kind="ExternalInput").ap()
    wq_d = nc.dram_tensor("wq", [2, 128, DKQ], bf16, kind="ExternalInput").ap()
    wk_d = nc.dram_tensor("wk", [2, 128, DKQ], bf16, kind="ExternalInput").ap()
    wv_d = nc.dram_tensor("wv", [2, 128, DV], bf16, kind="ExternalInput").ap()
    wp_d = nc.dram_tensor("wp", [4, 128, C], bf16, kind="ExternalInput").ap()
    bt_d = nc.dram_tensor("bt", [2, 128, 1], f32, kind="ExternalInput").ap()
    bp_d = nc.dram_tensor("bp", [2, 128, 1], f32, kind="ExternalInput").ap()
    gam_d = nc.dram_tensor("gamma", [2, 128, 1], f32, kind="ExternalInput").ap()
    bet_d = nc.dram_tensor("beta", [2, 128, 1], f32, kind="ExternalInput").ap()
    out_d = nc.dram_tensor("out", [2, 128, POS], f32, kind="ExternalOutput").ap()

    with tile.TileContext(nc) as tc:
        with (
            # bf16 stores feeding bf16 matmuls are deliberate; accumulation
            # stays fp32 in PSUM.
            nc.allow_low_precision("bf16 matmul operands by design"),
            tc.tile_pool(name="persist", bufs=1) as persist,
            tc.tile_pool(name="small", bufs=4) as small,
            tc.tile_pool(name="tmp", bufs=2) as tmp,
            tc.tile_pool(name="dram", bufs=1, space="DRAM") as dram,
        ):
            # ---- persistent SBUF tensors ----
            x_sb = persist.tile([128, 2, POS], f32, tag="x")
            xb_sb = persist.tile([128, 2, POS], f32, tag="xb")
            xn_sb = persist.tile([128, 2, POS], bf16, tag="xn")
            qt_sb = persist.tile([128, 4, POS], bf16, tag="qt")
            kt_sb = persist.tile([128, 4, POS], bf16, tag="kt")
            vp_sb = persist.tile([128, 8, DV + 1], bf16, tag="vp")  # [V | 1]
            a_sb = persist.tile([128, 4, POS], bf16, tag="a")
            wt_sb = persist.tile([128, 4, C], bf16, tag="wt")
            wq_sb = persist.tile([128, 2, DKQ], bf16, tag="wq")
            wk_sb = persist.tile([128, 2, DKQ], bf16, tag="wk")
            wv_sb = persist.tile([128, 2, DV], bf16, tag="wv")
            wp_sb = persist.tile([128, 4, C], bf16, tag="wp")
            t_sb = persist.tile([128, 4], bf16, tag="t")
            bt_sb = persist.tile([128, 2], f32, tag="bt")
            bp_sb = persist.tile([128, 2], f32, tag="bp")
            gam_sb = persist.tile([128, 2], f32, tag="gam")
            bet_sb = persist.tile([128, 2], f32, tag="bet")
            temb_sb = persist.tile([128, 2], f32, tag="temb")
            s_sb = persist.tile([128, 2], f32, tag="s")      # BN scale
            b_sb = persist.tile([128, 2], f32, tag="b")      # BN shift
            eps_sb = persist.tile([128, 1], f32, tag="eps")
            glob_sb = persist.tile([128, 2, 2], f32, tag="glob")

            cc_in = dram.tile([2, 128, 2], f32)
            cc_out = dram.tile([2, 128, 2], f32)

            # ---- input DMAs (issued up front; Tile overlaps them) ----
            for j in range(2):
                nc.sync.dma_start(x_sb[:, j, :], x_d[j])
            for a in range(4):
                nc.sync.dma_start(t_sb[:, a : a + 1], t_d[a])
                nc.sync.dma_start(wt_sb[:, a, :], wt_d[a])
                nc.sync.dma_start(wp_sb[:, a, :], wp_d[a])
            for j in range(2):
                nc.sync.dma_start(wq_sb[:, j, :], wq_d[j])
                nc.sync.dma_start(wk_sb[:, j, :], wk_d[j])
                nc.sync.dma_start(wv_sb[:, j, :], wv_d[j])
                nc.sync.dma_start(bt_sb[:, j : j + 1], bt_d[j])
                nc.sync.dma_start(bp_sb[:, j : j + 1], bp_d[j])
                nc.sync.dma_start(gam_sb[:, j : j + 1], gam_d[j])
                nc.sync.dma_start(bet_sb[:, j : j + 1], bet_d[j])

            nc.gpsimd.memset(eps_sb[:], EPS)
            nc.gpsimd.memset(vp_sb[:], 1.0)  # col DV stays 1; cols 0:DV overwritten

            # ---- phase 1: temb, xb, local BN stats, AllReduce ----
            with (
                tc.tile_pool(name="ps_big", bufs=2, space="PSUM") as ps_big,
                tc.tile_pool(name="ps_sm", bufs=2, space="PSUM") as ps_sm,
            ):
                for mc in range(2):
                    tp = ps_sm.tile([128, DV], f32, tag="sm")
                    for kc in range(4):
                        nc.tensor.matmul(
                            tp[:, 0:1],
                            wt_sb[:, kc, mc * 128 : (mc + 1) * 128],
                            t_sb[:, kc : kc + 1],
                            start=(kc == 0),
                            stop=(kc == 3),
                        )
                    # temb = relu(t @ Wt + bt)
                    nc.scalar.activation(
                        temb_sb[:, mc : mc + 1], tp[:, 0:1], AF.Relu,
                        bias=bt_sb[:, mc : mc + 1], scale=1.0,
                    )

                for j in range(2):
                    nc.vector.tensor_scalar_add(
                        xb_sb[:, j, :], x_sb[:, j, :], temb_sb[:, j : j + 1]
                    )
                    bnst = small.tile([128, 2, 6], f32, tag="bnst")
                    nc.vector.bn_stats(bnst[:, 0, :], xb_sb[:, j, 0:512])
                    nc.vector.bn_stats(bnst[:, 1, :], xb_sb[:, j, 512:1024])
                    mv = small.tile([128, 2], f32, tag="mv")
                    nc.vector.bn_aggr(mv[:], bnst[:])
                    # local sums: s1 = mean*1024 ; s2 = (var + mean^2)*1024
                    msq = small.tile([128, 1], f32, tag="msq")
                    nc.vector.tensor_tensor(msq[:], mv[:, 0:1], mv[:, 0:1], ALU.mult)
                    e2 = small.tile([128, 1], f32, tag="e2")
                    nc.vector.tensor_tensor(e2[:], mv[:, 1:2], msq[:], ALU.add)
                    loc = small.tile([128, 2], f32, tag="loc")
                    nc.vector.tensor_scalar_mul(loc[:, 0:1], mv[:, 0:1], float(POS))
                    nc.vector.tensor_scalar_mul(loc[:, 1:2], e2[:], float(POS))
                    nc.gpsimd.dma_start(cc_in[j], loc[:])

                nc.gpsimd.collective_compute(
                    "AllReduce",
                    ALU.add,
                    replica_groups=[list(range(N_CORES))],
                    ins=[cc_in.opt()],
                    outs=[cc_out.opt()],
                )
                for j in range(2):
                    nc.gpsimd.dma_start(glob_sb[:, j, :], cc_out[j])

                # ---- phase 2: global stats -> per-channel affine; xn ----
                for j in range(2):
                    mean = small.tile([128, 1], f32, tag="mean")
                    nc.vector.tensor_scalar_mul(mean[:], glob_sb[:, j, 0:1], 1.0 / CNT)
                    ex2 = small.tile([128, 1], f32, tag="ex2")
                    nc.vector.tensor_scalar_mul(ex2[:], glob_sb[:, j, 1:2], 1.0 / CNT)
                    msq2 = small.tile([128, 1], f32, tag="msq2")
                    nc.vector.tensor_tensor(msq2[:], mean[:], mean[:], ALU.mult)
                    var = small.tile([128, 1], f32, tag="var")
                    nc.vector.tensor_tensor(var[:], ex2[:], msq2[:], ALU.subtract)
                    # rstd = exp(-0.5 * ln(var + eps)) — stays in the
                    # natural_log_exp_and_others table set (shared with softmax)
                    lnv = small.tile([128, 1], f32, tag="lnv")
                    nc.scalar.activation(lnv[:], var[:], AF.Ln, bias=eps_sb[:])
                    rstd = small.tile([128, 1], f32, tag="rstd")
                    nc.scalar.activation(rstd[:], lnv[:], AF.Exp, scale=-0.5)
                    nc.vector.tensor_tensor(
                        s_sb[:, j : j + 1], gam_sb[:, j : j + 1], rstd[:], ALU.mult
                    )
                    mb = small.tile([128, 1], f32, tag="mb")
                    nc.vector.tensor_tensor(mb[:], mean[:], s_sb[:, j : j + 1], ALU.mult)
                    nc.vector.tensor_tensor(
                        b_sb[:, j : j + 1], bet_sb[:, j : j + 1], mb[:], ALU.subtract
                    )
                    # xn = xb * s + b  (bf16 out for the projection matmuls)
                    nc.vector.tensor_scalar(
                        xn_sb[:, j, :], xb_sb[:, j, :],
                        s_sb[:, j : j + 1], b_sb[:, j : j + 1],
                        ALU.mult, ALU.add,
                    )

                # ---- phase 3: Q_T, K_T, V' ----
                for dst, w in ((qt_sb, wq_sb), (kt_sb, wk_sb)):
                    for mj in range(4):
                        qp = ps_big.tile([128, POS], f32, tag="big")
                        for kc in range(2):
                            for hf in range(2):
                                nc.tensor.matmul(
                                    qp[:, hf * 512 : (hf + 1) * 512],
                                    w[:, kc, mj * 128 : (mj + 1) * 128],
                                    xn_sb[:, kc, hf * 512 : (hf + 1) * 512],
                                    start=(kc == 0),
                                    stop=(kc == 1),
                                )
                        nc.vector.tensor_copy(dst[:, mj, :], qp[:])
                for pc in range(8):
                    vps = ps_sm.tile([128, DV], f32, tag="sm")
                    for kc in range(2):
                        nc.tensor.matmul(
                            vps[:],
                            xn_sb[:, kc, pc * 128 : (pc + 1) * 128],
                            wv_sb[:, kc, :],
                            start=(kc == 0),
                            stop=(kc == 1),
                        )
                    nc.vector.tensor_copy(vp_sb[:, pc, 0:DV], vps[:])

            # ---- phase 4: attention, one head pair at a time ----
            with (
                tc.tile_pool(name="ps_st", bufs=2, space="PSUM") as ps_st,
                tc.tile_pool(name="ps_av", bufs=2, space="PSUM") as ps_av,
                tc.tile_pool(name="pp", bufs=3) as pp,
                tc.tile_pool(name="rp", bufs=2) as rp,
                tc.tile_pool(name="dp", bufs=2) as dp,
                tc.tile_pool(name="rdram", bufs=2, space="DRAM") as rdram,
            ):
                for j in range(4):  # head pair (2j, 2j+1); also the Q/K chunk
                    avs = {}
                    for o in (0, 64):  # head offset within chunk
                        avs[o] = ps_av.tile([128, POS], f32, tag="av", name=f"av{j}_{o}")
                    for kc in range(8):
                        for o in (0, 64):
                            st = ps_st.tile([128, POS], f32, tag="st")
                            for hf in range(2):
                                nc.tensor.matmul(
                                    st[:, hf * 512 : (hf + 1) * 512],
                                    kt_sb[o : o + DH, j, kc * 128 : (kc + 1) * 128],
                                    qt_sb[o : o + DH, j, hf * 512 : (hf + 1) * 512],
                                    start=True,
                                    stop=True,
                                )
                            p = pp.tile([128, POS], bf16, tag="p")
                            nc.scalar.activation(p[:], st[:], AF.Exp, scale=SCALE)
                            for hf in range(2):
                                nc.tensor.matmul(
                                    avs[o][0 : DV + 1, hf * 512 : (hf + 1) * 512],
                                    vp_sb[:, kc, :],
                                    p[:, hf * 512 : (hf + 1) * 512],
                                    start=(kc == 0),
                                    stop=(kc == 7),
                                )
                    for o in (0, 64):
                        av = avs[o]
                        rr = rp.tile([128, POS], bf16, tag="r")
                        nc.vector.reciprocal(rr[64:65, :], av[64:65, :])
                        # broadcast 1/denom across the 64 dv partitions: DVE
                        # can't read two PSUM srcs and engines can't cross
                        # partitions, so bounce through DRAM and re-read with
                        # a stride-0 partition AP (2 KB + 128 KB DMAs, fully
                        # pipelined behind the next head's compute)
                        rd = rdram.tile([1, POS], bf16, tag="rd")
                        nc.sync.dma_start(rd[:], rr[64:65, :])
                        d_sb = dp.tile([DV, POS], bf16, tag="d")
                        nc.sync.dma_start(d_sb[:], rd.partition_broadcast(DV))
                        # A chunk j rows [o, o+64) = AV / denom
                        nc.vector.tensor_tensor(
                            a_sb[o : o + DV, j, :], av[0:DV, :], d_sb[:], ALU.mult
                        )

            # ---- phase 5: projection + bias + residual ----
            with tc.tile_pool(name="ps_pr", bufs=2, space="PSUM") as ps_pr:
                for mj in range(2):
                    op = ps_pr.tile([128, POS], f32, tag="pr")
                    for kc in range(4):
                        for hf in range(2):
                            nc.tensor.matmul(
                                op[:, hf * 512 : (hf + 1) * 512],
                                wp_sb[:, kc, mj * 128 : (mj + 1) * 128],
                                a_sb[:, kc, hf * 512 : (hf + 1) * 512],
                                start=(kc == 0),
                                stop=(kc == 3),
                            )
                    t1 = tmp.tile([128, POS], f32, tag="t1")
                    nc.vector.tensor_tensor(t1[:], op[:], x_sb[:, mj, :], ALU.add)
                    t2 = tmp.tile([128, POS], f32, tag="t2")
                    nc.vector.tensor_scalar_add(t2[:], t1[:], bp_sb[:, mj : mj + 1])
                    nc.sync.dma_start(out_d[mj], t2[:])

    nc.compile()
    return nc


def _get_program():
    if "nc" not in _CACHE:
        _CACHE["nc"] = _build_program()
    return _CACHE["nc"]


def _make_in_maps(x, t, Wt, bt, Wq, Wk, Wv, Wp, bp, gamma, beta):
    import ml_dtypes

    f = np.float32
    bf = ml_dtypes.bfloat16
    x = np.ascontiguousarray(x, dtype=f)
    t = np.ascontiguousarray(t, dtype=f)
    # Wp rows are indexed dv*H + h in the reference (out.reshape flattens
    # [dv, head]); the kernel builds A head-major (h*DV + dv), so permute.
    Wp_p = np.ascontiguousarray(
        np.asarray(Wp, dtype=f).reshape(DV, H, C).transpose(1, 0, 2).reshape(H * DV, C)
    )
    shared = {
        "wt": np.asarray(Wt, f).reshape(4, 128, C).astype(bf),
        "wq": np.asarray(Wq, f).reshape(2, 128, DKQ).astype(bf),
        "wk": np.asarray(Wk, f).reshape(2, 128, DKQ).astype(bf),
        "wv": np.asarray(Wv, f).reshape(2, 128, DV).astype(bf),
        "wp": Wp_p.reshape(4, 128, C).astype(bf),
        "bt": np.ascontiguousarray(np.asarray(bt, f).reshape(2, 128, 1)),
        "bp": np.ascontiguousarray(np.asarray(bp, f).reshape(2, 128, 1)),
        "gamma": np.ascontiguousarray(np.asarray(gamma, f).reshape(2, 128, 1)),
        "beta": np.ascontiguousarray(np.asarray(beta, f).reshape(2, 128, 1)),
    }
    in_maps = []
    for n in range(N_CORES):
        m = dict(shared)
        m["x"] = np.ascontiguousarray(x[n].reshape(2, 128, POS))
        m["t"] = t[n].reshape(4, 128, 1).astype(bf)
        in_maps.append(m)
    return in_maps


def kernel(**inputs) -> np.ndarray:
    from concourse.bass_utils import run_bass_kernel_spmd

    nc = _get_program()
    in_maps = _make_in_maps(**inputs)
    trace = os.environ.get("KERNEL_TRACE", "0") == "1"
    res = run_bass_kernel_spmd(nc, in_maps, list(range(N_CORES)), trace=trace)
    if trace:
        _CACHE["last_results"] = res
    out = np.empty((N_CORES, C, 32, 32), dtype=np.float32)
    for n in range(N_CORES):
        out[n] = res.results[n]["out"].reshape(C, 32, 32)
    return out
